# revision 1
# baseline (speedup 1.0000x reference)
"""MoE (top-1 routing, E=8) Trainium2 Bass kernel — H-sharded merged-weight.

out = x @ (Ws + We[e]).T + (bs + be[e])   (top-1 partition => one matmul)

Sharding: each core owns a 512-wide slice of H and computes it for ALL
tokens, which are globally sorted by expert into `nt` 128-token tiles
(nt = sum_e ceil(count_e/128) ~ 131).  Every core runs the identical
static schedule; the only per-core difference is which weight/bias
columns are bound (pure input binding).  Per tile: 8 accumulating
matmuls (K=1024, N=512) against the resident merged-weight slice of
the tile's expert, a DVE bias add, and a 128 KB row store.

Device residency per core: all 8 experts' merged-weight slices
(8 x 1.05 MB) + bias (1 MB) -- the first matmul is gated on just
1.05 MB of DMA, so there is no weight race at startup.
"""

import sys

sys.path.insert(0, "/opt/trn_rl_repo")

import numpy as np

import concourse.bass as bass
import concourse.mybir as mybir
from concourse.tile import TileContext

N, D, H, E = 16384, 1024, 4096, 8
N_CORES = 8
KC = D // 128
HS = H // N_CORES      # 512: per-core H slice

F16 = mybir.dt.float16
F32 = mybir.dt.float32

MAX_WAITS = 1


def split_long_waits(nc, max_w: int = MAX_WAITS):
    """walrus TPB_CTRL codegen rejects instructions with multiple sync
    waits; move excess waits onto same-engine NoOps."""
    n_fix = 0
    for f in nc.m.functions:
        for bb in f.blocks:
            insts = bb.instructions
            new_list = []
            changed = False
            for inst in insts:
                si = inst.sync_info
                if si is not None and len(si.on_wait) > max_w:
                    w = list(si.on_wait)
                    k = 0
                    while len(w) > max_w:
                        chunk, w = w[:max_w], w[max_w:]
                        nop = mybir.InstNoOp(
                            name=f"{inst.name}_waitsplit_{k}",
                            engine=inst.engine,
                            sync_info=mybir.SyncInfo(on_wait=chunk, on_update=[]),
                            bass_nofuse=True,
                        )
                        new_list.append(nop)
                        k += 1
                    inst.sync_info = mybir.SyncInfo(
                        on_wait=w, on_update=list(si.on_update)
                    )
                    n_fix += 1
                    changed = True
                new_list.append(inst)
            if changed:
                bb.instructions = new_list
    return n_fix


# ----------------------------------------------------------------------------
# device program (static schedule = expert id per token tile)
# ----------------------------------------------------------------------------


def build_program(sched: tuple, fix_waits: bool = True):
    nt = len(sched)
    nc = bass.Bass()

    # [e, p, k*512+j] = (Ws+We[e]).T[k*128+p, core*512+j]
    w_d = nc.declare_dram_parameter("w16", [E, 128, KC * HS], F16, isOutput=False)
    b_d = nc.declare_dram_parameter("b16", [128, E * HS], F16, isOutput=False)
    xg_d = nc.declare_dram_parameter("xg16", [nt, 128, KC * 128], F16, isOutput=False)
    out_d = nc.declare_dram_parameter("out", [nt * 128, HS], F16, isOutput=True)

    first_use = []
    seen = set()
    for e in sched:
        if e not in seen:
            seen.add(e)
            first_use.append(e)
    rest = [e for e in range(E) if e not in seen]

    with TileContext(nc) as tc:
        with (
            tc.tile_pool(name="wres", bufs=1) as wpool,
            tc.tile_pool(name="xstream", bufs=6) as xpool,
            tc.tile_pool(name="ostage", bufs=4) as opool,
            tc.tile_pool(name="ps", bufs=4, space="PSUM") as pspool,
        ):
            w = wpool.tile([128, E, KC * HS], F16, tag="w")
            b = wpool.tile([128, E * HS], F16, tag="b")

            # The sync queue's DMA ring comes up first (~9 us) vs
            # gpsimd's (~13 us): put the first token tiles + first
            # weight slice there so matmul 0 can issue ASAP.  Bias
            # rides the (otherwise idle-at-start) scalar queue; the
            # token stream runs on gpsimd.
            n_pre = min(2, nt)
            xts = {}
            for t in range(n_pre):
                xt = xpool.tile([128, KC * 128], F16, tag="xt", name=f"xt{t}")
                nc.sync.dma_start(out=xt[:, :], in_=xg_d[t, :, :])
                xts[t] = xt
            nc.scalar.dma_start(out=b[:, :], in_=b_d[:, :])
            for e in first_use + rest:
                nc.sync.dma_start(out=w[:, e, :], in_=w_d[e, :, :])
            load_at = {}

            for t in range(nt):
                e = sched[t]
                for el, kind, k in load_at.get(t, ()):
                    if kind == "b":
                        nc.gpsimd.dma_start(
                            out=b[:, el * HS : (el + 1) * HS],
                            in_=b_d[:, el * HS : (el + 1) * HS],
                        )
                    elif kind == "w":
                        nc.gpsimd.dma_start(
                            out=w[:, el, k * HS : (k + 1) * HS],
                            in_=w_d[el, :, k * HS : (k + 1) * HS],
                        )
                    else:
                        nc.gpsimd.dma_start(out=w[:, el, :], in_=w_d[el, :, :])
                if t in xts:
                    xt = xts[t]
                else:
                    xt = xpool.tile([128, KC * 128], F16, tag="xt")
                    nc.gpsimd.dma_start(out=xt[:, :], in_=xg_d[t, :, :])
                ot = opool.tile([128, HS], F16, tag="ot")
                ps = pspool.tile([128, HS], F32, tag="ps")
                for k in range(KC):
                    nc.tensor.matmul(
                        ps[:, :],
                        lhsT=xt[:, k * 128 : (k + 1) * 128],
                        rhs=w[:, e, k * HS : (k + 1) * HS],
                        start=(k == 0),
                        stop=(k == KC - 1),
                    )
                nc.vector.tensor_add(
                    out=ot[:, :],
                    in0=ps[:, :],
                    in1=b[:, e * HS : (e + 1) * HS],
                )
                nc.scalar.dma_start(
                    out=out_d[t * 128 : (t + 1) * 128, :], in_=ot[:, :]
                )

    if fix_waits:
        split_long_waits(nc)
    return nc


# ----------------------------------------------------------------------------
# host-side routing / packing / scatter
# ----------------------------------------------------------------------------


def route(te):
    """-> (sched tuple, tokens [nt*128] with -1 pads)."""
    sched = []
    toks = []
    for e in range(E):
        ids = np.nonzero(te == e)[0]
        if len(ids) == 0:
            continue
        nt_e = int(np.ceil(len(ids) / 128))
        pad = np.full(nt_e * 128, -1, dtype=np.int64)
        pad[: len(ids)] = ids
        sched += [e] * nt_e
        toks.append(pad)
    return tuple(sched), np.concatenate(toks)


def _tile_x(x16, toks):
    tk = np.where(toks < 0, 0, toks)
    xt = x16[tk]  # [nt*128, D]
    m = len(tk) // 128
    return np.ascontiguousarray(
        xt.reshape(m, 128, KC, 128).transpose(0, 3, 2, 1).reshape(m, 128, KC * 128)
    )


def make_in_maps(x, Ws, bs, We, be, toks):
    x16 = x.astype(np.float16)
    xg = _tile_x(x16, toks)
    in_maps = []
    for c in range(N_CORES):
        ws = np.empty((E, 128, KC * HS), dtype=np.float16)
        bias = np.empty((128, E * HS), dtype=np.float16)
        for e in range(E):
            WT = (Ws + We[e]).T[:, c * HS : (c + 1) * HS]  # [D, HS] fp32
            ws[e] = (
                WT.reshape(KC, 128, HS).transpose(1, 0, 2).reshape(128, KC * HS)
            ).astype(np.float16)
            bias[:, e * HS : (e + 1) * HS] = (
                (bs + be[e])[c * HS : (c + 1) * HS].astype(np.float16)
            )
        in_maps.append({"w16": ws, "b16": bias, "xg16": xg})
    return in_maps


def scatter_out(results, toks):
    out = np.empty((N, H), dtype=np.float32)
    valid = toks >= 0
    tv = toks[valid]
    for c in range(N_CORES):
        rows = results[c]["out"]  # [nt*128, HS] fp16
        out[tv, c * HS : (c + 1) * HS] = rows[valid].astype(np.float32)
    return out


# ----------------------------------------------------------------------------
# entry point
# ----------------------------------------------------------------------------

_PROGRAM_CACHE = {}


def _get_program(sched):
    if sched not in _PROGRAM_CACHE:
        _PROGRAM_CACHE[sched] = build_program(sched)
    return _PROGRAM_CACHE[sched]


def prepare(x, Ws, bs, We, be, Wr, br):
    x = np.asarray(x, dtype=np.float32)
    Ws = np.asarray(Ws, dtype=np.float32)
    bs = np.asarray(bs, dtype=np.float32)
    We = np.asarray(We, dtype=np.float32)
    be = np.asarray(be, dtype=np.float32)
    Wr = np.asarray(Wr, dtype=np.float32)
    br = np.asarray(br, dtype=np.float32)
    assert x.shape == (N, D)

    logits = x @ Wr.T + br
    te = np.argmax(logits, axis=-1)
    sched, toks = route(te)
    nc = _get_program(sched)
    in_maps = make_in_maps(x, Ws, bs, We, be, toks)
    return nc, in_maps, toks


def finish(results, toks):
    return scatter_out(results, toks)


def kernel(x, Ws, bs, We, be, Wr, br):
    from concourse.bass_utils import run_bass_kernel_spmd

    nc, in_maps, toks = prepare(x, Ws, bs, We, be, Wr, br)
    res = run_bass_kernel_spmd(nc, in_maps, list(range(N_CORES)))
    return finish(res.results, toks)



# revision 2
# speedup vs baseline: 1.0249x; 1.0249x over previous
"""MoE (top-1 routing, E=8) Trainium2 Bass kernel — H-sharded merged-weight.

out = x @ (Ws + We[e]).T + (bs + be[e])   (top-1 partition => one matmul)

Sharding: each core owns a 512-wide slice of H and computes it for ALL
tokens, which are globally sorted by expert into `nt` 128-token tiles
(nt = sum_e ceil(count_e/128) ~ 131).  Every core runs the identical
static schedule; the only per-core difference is which weight/bias
columns are bound (pure input binding).  Per tile: 8 accumulating
matmuls (K=1024, N=512) against the resident merged-weight slice of
the tile's expert, a DVE bias add, and a 128 KB row store.

Schedule notes (v2):
- ~34 dummy matmuls on a zeroed SBUF tile at t=0 warm the PE HAM clock
  gate (1.2 -> 2.4 GHz) during the ~10 us DMA-ring bringup, so the
  first real matmul runs at full rate.
- Expert weight slices are streamed just-in-time (8 x 128 KB chunks on
  the gpsimd queue, paced by the x-stream FIFO, landing ~LEAD tiles
  before the expert's first token tile).  The old upfront 8 MB blast
  saturated HBM during the first 50 us, starving the x stream and
  re-throttling the clock gate.
- Tile 0's x and weight arrive as interleaved 32/128 KB chunks on the
  sync queue so matmul 0 issues as soon as the ring is up.
"""

import sys

sys.path.insert(0, "/opt/trn_rl_repo")

import numpy as np

import concourse.bass as bass
import concourse.mybir as mybir
from concourse.tile import TileContext

N, D, H, E = 16384, 1024, 4096, 8
N_CORES = 8
KC = D // 128
HS = H // N_CORES      # 512: per-core H slice

F16 = mybir.dt.float16
F32 = mybir.dt.float32

MAX_WAITS = 1
N_DUMMY = 34           # PE warmup matmuls (~9.7 us: 8 cold + 26 warm)
LEAD = 12              # weight-chunk lead, in tiles, before expert first use
N_PRE_SYNC = 4         # x tiles carried on the sync queue at startup


def split_long_waits(nc, max_w: int = MAX_WAITS):
    """walrus TPB_CTRL codegen rejects instructions with multiple sync
    waits; move excess waits onto same-engine NoOps."""
    n_fix = 0
    for f in nc.m.functions:
        for bb in f.blocks:
            insts = bb.instructions
            new_list = []
            changed = False
            for inst in insts:
                si = inst.sync_info
                if si is not None and len(si.on_wait) > max_w:
                    w = list(si.on_wait)
                    k = 0
                    while len(w) > max_w:
                        chunk, w = w[:max_w], w[max_w:]
                        nop = mybir.InstNoOp(
                            name=f"{inst.name}_waitsplit_{k}",
                            engine=inst.engine,
                            sync_info=mybir.SyncInfo(on_wait=chunk, on_update=[]),
                            bass_nofuse=True,
                        )
                        new_list.append(nop)
                        k += 1
                    inst.sync_info = mybir.SyncInfo(
                        on_wait=w, on_update=list(si.on_update)
                    )
                    n_fix += 1
                    changed = True
                new_list.append(inst)
            if changed:
                bb.instructions = new_list
    return n_fix


# ----------------------------------------------------------------------------
# device program (static schedule = expert id per token tile)
# ----------------------------------------------------------------------------


def build_program(sched: tuple, fix_waits: bool = True):
    nt = len(sched)
    nc = bass.Bass()

    # [e, p, k*512+j] = (Ws+We[e]).T[k*128+p, core*512+j]
    w_d = nc.declare_dram_parameter("w16", [E, 128, KC * HS], F16, isOutput=False)
    b_d = nc.declare_dram_parameter("b16", [128, E * HS], F16, isOutput=False)
    xg_d = nc.declare_dram_parameter("xg16", [nt, 128, KC * 128], F16, isOutput=False)
    out_d = nc.declare_dram_parameter("out", [nt * 128, HS], F16, isOutput=True)

    first_tile = {}
    for t, e in enumerate(sched):
        if e not in first_tile:
            first_tile[e] = t
    experts_used = sorted(first_tile, key=first_tile.get)

    # JIT weight-chunk schedule: expert e's k-th 128 KB chunk rides the
    # gpsimd queue at tile slot ~ (first_use - LEAD + k), paced by the
    # x-stream FIFO ahead of it.  First expert loads on sync at startup.
    load_at = {}
    for e in experts_used[1:]:
        f = first_tile[e]
        for k in range(KC):
            slot = max(1, min(f - LEAD + k, f - 1))
            load_at.setdefault(slot, []).append((e, k))

    with TileContext(nc) as tc:
        with (
            tc.tile_pool(name="wres", bufs=1) as wpool,
            tc.tile_pool(name="xstream", bufs=10) as xpool,
            tc.tile_pool(name="ostage", bufs=4) as opool,
            tc.tile_pool(name="ps", bufs=4, space="PSUM") as pspool,
            tc.tile_pool(name="psdmy", bufs=1, space="PSUM") as dmypool,
        ):
            w = wpool.tile([128, E, KC * HS], F16, tag="w")
            b = wpool.tile([128, E * HS], F16, tag="b")

            # PE warmup: matmuls on a zeroed tile, result never read.
            dmy = wpool.tile([128, 512], F16, tag="dmy")
            dps = dmypool.tile([128, 512], F32, tag="dps")
            nc.vector.memset(dmy[:, :], 0.0)
            for _ in range(N_DUMMY):
                nc.tensor.matmul(
                    dps[:, :],
                    lhsT=dmy[:, 0:128],
                    rhs=dmy[:, :],
                    start=True,
                    stop=True,
                )

            # Startup on the sync queue (HWDGE ring, up first): tile 0's
            # x and first expert's weights interleaved in k-chunks, then
            # a few whole x tiles.  Bias rides the scalar queue.
            e0 = sched[0]
            xts = {}
            n_pre = min(N_PRE_SYNC, nt)
            xt0 = xpool.tile([128, KC * 128], F16, tag="xt", name="xt0")
            xts[0] = xt0
            for k in range(KC):
                nc.sync.dma_start(
                    out=xt0[:, k * 128 : (k + 1) * 128],
                    in_=xg_d[0, :, k * 128 : (k + 1) * 128],
                )
                nc.sync.dma_start(
                    out=w[:, e0, k * HS : (k + 1) * HS],
                    in_=w_d[e0, :, k * HS : (k + 1) * HS],
                )
            for t in range(1, n_pre):
                xt = xpool.tile([128, KC * 128], F16, tag="xt", name=f"xt{t}")
                nc.sync.dma_start(out=xt[:, :], in_=xg_d[t, :, :])
                xts[t] = xt
            nc.scalar.dma_start(out=b[:, :], in_=b_d[:, :])

            for t in range(nt):
                e = sched[t]
                for el, k in load_at.get(t, ()):
                    nc.gpsimd.dma_start(
                        out=w[:, el, k * HS : (k + 1) * HS],
                        in_=w_d[el, :, k * HS : (k + 1) * HS],
                    )
                if t in xts:
                    xt = xts[t]
                else:
                    xt = xpool.tile([128, KC * 128], F16, tag="xt")
                    nc.gpsimd.dma_start(out=xt[:, :], in_=xg_d[t, :, :])
                ot = opool.tile([128, HS], F16, tag="ot")
                ps = pspool.tile([128, HS], F32, tag="ps")
                for k in range(KC):
                    nc.tensor.matmul(
                        ps[:, :],
                        lhsT=xt[:, k * 128 : (k + 1) * 128],
                        rhs=w[:, e, k * HS : (k + 1) * HS],
                        start=(k == 0),
                        stop=(k == KC - 1),
                    )
                nc.vector.tensor_add(
                    out=ot[:, :],
                    in0=ps[:, :],
                    in1=b[:, e * HS : (e + 1) * HS],
                )
                nc.scalar.dma_start(
                    out=out_d[t * 128 : (t + 1) * 128, :], in_=ot[:, :]
                )

    if fix_waits:
        split_long_waits(nc)
    return nc


# ----------------------------------------------------------------------------
# host-side routing / packing / scatter
# ----------------------------------------------------------------------------


def route(te):
    """-> (sched tuple, tokens [nt*128] with -1 pads)."""
    sched = []
    toks = []
    for e in range(E):
        ids = np.nonzero(te == e)[0]
        if len(ids) == 0:
            continue
        nt_e = int(np.ceil(len(ids) / 128))
        pad = np.full(nt_e * 128, -1, dtype=np.int64)
        pad[: len(ids)] = ids
        sched += [e] * nt_e
        toks.append(pad)
    return tuple(sched), np.concatenate(toks)


def _tile_x(x16, toks):
    tk = np.where(toks < 0, 0, toks)
    xt = x16[tk]  # [nt*128, D]
    m = len(tk) // 128
    return np.ascontiguousarray(
        xt.reshape(m, 128, KC, 128).transpose(0, 3, 2, 1).reshape(m, 128, KC * 128)
    )


def make_in_maps(x, Ws, bs, We, be, toks):
    x16 = x.astype(np.float16)
    xg = _tile_x(x16, toks)
    in_maps = []
    for c in range(N_CORES):
        ws = np.empty((E, 128, KC * HS), dtype=np.float16)
        bias = np.empty((128, E * HS), dtype=np.float16)
        for e in range(E):
            WT = (Ws + We[e]).T[:, c * HS : (c + 1) * HS]  # [D, HS] fp32
            ws[e] = (
                WT.reshape(KC, 128, HS).transpose(1, 0, 2).reshape(128, KC * HS)
            ).astype(np.float16)
            bias[:, e * HS : (e + 1) * HS] = (
                (bs + be[e])[c * HS : (c + 1) * HS].astype(np.float16)
            )
        in_maps.append({"w16": ws, "b16": bias, "xg16": xg})
    return in_maps


def scatter_out(results, toks):
    out = np.empty((N, H), dtype=np.float32)
    valid = toks >= 0
    tv = toks[valid]
    for c in range(N_CORES):
        rows = results[c]["out"]  # [nt*128, HS] fp16
        out[tv, c * HS : (c + 1) * HS] = rows[valid].astype(np.float32)
    return out


# ----------------------------------------------------------------------------
# entry point
# ----------------------------------------------------------------------------

_PROGRAM_CACHE = {}


def _get_program(sched):
    if sched not in _PROGRAM_CACHE:
        _PROGRAM_CACHE[sched] = build_program(sched)
    return _PROGRAM_CACHE[sched]


def prepare(x, Ws, bs, We, be, Wr, br):
    x = np.asarray(x, dtype=np.float32)
    Ws = np.asarray(Ws, dtype=np.float32)
    bs = np.asarray(bs, dtype=np.float32)
    We = np.asarray(We, dtype=np.float32)
    be = np.asarray(be, dtype=np.float32)
    Wr = np.asarray(Wr, dtype=np.float32)
    br = np.asarray(br, dtype=np.float32)
    assert x.shape == (N, D)

    logits = x @ Wr.T + br
    te = np.argmax(logits, axis=-1)
    sched, toks = route(te)
    nc = _get_program(sched)
    in_maps = make_in_maps(x, Ws, bs, We, be, toks)
    return nc, in_maps, toks


def finish(results, toks):
    return scatter_out(results, toks)


def kernel(x, Ws, bs, We, be, Wr, br):
    from concourse.bass_utils import run_bass_kernel_spmd

    nc, in_maps, toks = prepare(x, Ws, bs, We, be, Wr, br)
    res = run_bass_kernel_spmd(nc, in_maps, list(range(N_CORES)))
    return finish(res.results, toks)
